# revision 1
# baseline (speedup 1.0000x reference)
"""GAT layer (LayerNorm -> GATConv(heads=1) -> residual ReLU) on 8 trn2 NeuronCores.

Sharding: destination-node parallel. Each core owns N/8 nodes, computes the
fused LN+linear transform for its shard (emitting bf16 rows
[feat(128) | a_src] at 512 B pitch plus an f32 a_dst column per dst block),
AllGathers the bf16 node table, then processes the edges whose destination
falls in its shard.

Edges are bucketed by (dst block, src half) into 128-slot tiles. Per tile:
the source rows are fetched with dma_gather (512 B elems, 4 SWDGE queues
round-robin so transfers overlap), a_dst is broadcast to slots with a
1-column matmul against a host-shipped transposed one-hot (ohT), attention
ee = exp(leakyrelu(a_src + a_dst)) runs on DVE+ACT, sw = onehot * ee, and the
scatter is the transposed matmul psB[feat, dst] += gf_feat^T-contract-sw with
a ones-column matmul giving the softmax denominator. Per block: transpose
back, normalize, add residual, ReLU.
"""

import numpy as np
import ml_dtypes

import concourse.bacc as bacc
import concourse.mybir as mybir
import concourse.tile as tile
from concourse.bass_utils import run_bass_kernel_spmd

F32 = mybir.dt.float32
BF16 = mybir.dt.bfloat16
I16 = mybir.dt.int16
AX = mybir.AxisListType
OP = mybir.AluOpType
AF = mybir.ActivationFunctionType

N = 50000
D = 128
E = 600000
NCORES = 8
SHARD = N // NCORES            # 6250
NBLK = (SHARD + 127) // 128    # 49 dst blocks per core
PAD_SHARD = NBLK * 128         # 6272
LAST_ROWS = SHARD - (NBLK - 1) * 128  # 106
ROW = 128                      # bf16 cols per table row (256 B pitch)
HALF = 32768                   # int16 index split point for the global table
NEG_SLOPE = 0.2
LN_EPS = 1e-5
GBLK = 3                       # dst blocks per group
NQ = 4                         # SWDGE queues
import os as _os
NQ = int(_os.environ.get("KNQ", "4"))
DEBUG_EE1 = _os.environ.get("DEBUG_EE1") == "1"   # force ee = 1 (bisection aid)
DEBUG_RAW = _os.environ.get("DEBUG_RAW") == "1"   # out = [denom | numerator] (bisection aid)
DEBUG_XP = _os.environ.get("DEBUG_XP") == "1"     # out = own xp rows (bisection aid)
DEBUG_GF = _os.environ.get("DEBUG_GF") == "1"     # out = gathered first-tile rows (bisection aid)


def _build_program(tlo, thi):
    """One SPMD program; per-core behaviour differs only through its inputs."""
    nc = bacc.Bacc("TRN2", num_devices=NCORES, debug=False, num_swdge_queues=NQ)

    CB = sum(tlo) + sum(thi)   # total column-block tiles per core

    x_shard = nc.dram_tensor("x_shard", [PAD_SHARD, D], F32, kind="ExternalInput")
    wext = nc.dram_tensor("wext", [D, 129], F32, kind="ExternalInput")
    vsrcb = nc.dram_tensor("vsrcb", [128, 128], BF16, kind="ExternalInput")
    c2b = nc.dram_tensor("c2b", [128, 129], F32, kind="ExternalInput")
    ident = nc.dram_tensor("ident", [128, 128], F32, kind="ExternalInput")
    feat_idx = nc.dram_tensor("feat_idx", [128, CB * 8], I16, kind="ExternalInput")
    oh_in = nc.dram_tensor("oh_in", [128, CB * 128], BF16, kind="ExternalInput")
    ohT_in = nc.dram_tensor("ohT_in", [128, CB * 128], BF16, kind="ExternalInput")
    out_shard = nc.dram_tensor("out_shard", [SHARD, D], F32, kind="ExternalOutput")

    # group structure: per group, lo tiles of its blocks then hi tiles
    groups = []  # (tiles, cb0) ; tiles = list of (block, half)
    cb0 = 0
    for g0 in range(0, NBLK, GBLK):
        blocks = list(range(g0, min(NBLK, g0 + GBLK)))
        tiles = []
        for b in blocks:
            tiles += [(b, 0)] * tlo[b]
        nlo = len(tiles)
        for b in blocks:
            tiles += [(b, 1)] * thi[b]
        groups.append((tiles, cb0, nlo))
        cb0 += len(tiles)
    assert cb0 == CB
    TG_MAX = max(len(t) for t, _, _ in groups)

    # first/last tile index (within CB) per block, for psum start/stop
    first_cb = {}
    last_cb = {}
    for tiles, c0, _ in groups:
        for j, (b, hf) in enumerate(tiles):
            cb = c0 + j
            first_cb.setdefault(b, cb)
            last_cb[b] = cb

    with tile.TileContext(nc) as tc:
        with (
            tc.tile_pool(name="dram", bufs=1, space="DRAM") as dram,
            tc.tile_pool(name="consts", bufs=1) as cpool,
            tc.tile_pool(name="xres", bufs=1) as xpool,
        ):
            xp_chunk = dram.tile([SHARD, ROW], BF16)
            xp_full = dram.tile([N, ROW], BF16, addr_space="Shared")

            ident_sb = cpool.tile([128, 128], F32)
            nc.sync.dma_start(ident_sb[:], ident[:, :])
            identb_sb = cpool.tile([128, 128], BF16)
            nc.vector.tensor_copy(identb_sb[:], ident_sb[:])
            wext_sb = cpool.tile([D, 129], F32)
            nc.sync.dma_start(wext_sb[:], wext[:, :])
            c2b_sb = cpool.tile([128, 129], F32)
            nc.sync.dma_start(c2b_sb[:], c2b[:, :])
            vsrc_sb = cpool.tile([128, 128], BF16)
            nc.sync.dma_start(vsrc_sb[:], vsrcb[:, :])
            eps_sb = cpool.tile([128, 1], F32)
            nc.vector.memset(eps_sb[:], LN_EPS)
            fidx_sb = cpool.tile([128, CB * 8], I16)
            nc.sync.dma_start(fidx_sb[:], feat_idx[:, :])
            adst_all = cpool.tile([128, NBLK], BF16)

            x_tiles = []
            for i in range(NBLK):
                xt = xpool.tile([128, D], F32, tag=f"xres{i}")
                nc.sync.dma_start(xt[:], x_shard[i * 128 : (i + 1) * 128, :])
                x_tiles.append(xt)

            # ---------------- Phase A: node transform on own shard ---------
            with (
                tc.tile_pool(name="a_small", bufs=8) as spool,
                tc.tile_pool(name="a_sq", bufs=2) as sqpool,
                tc.tile_pool(name="a_xnp", bufs=3) as xnppool,
                tc.tile_pool(name="a_xnpT", bufs=3) as xnptpool,
                tc.tile_pool(name="a_xpe", bufs=3) as xpepool,
                tc.tile_pool(name="a_ps_t", bufs=2, space="PSUM") as psa,
                tc.tile_pool(name="a_ps_m", bufs=2, space="PSUM") as psb,
            ):
                for i in range(NBLK):
                    xt = x_tiles[i]
                    rows = 128 if i < NBLK - 1 else LAST_ROWS
                    sumx = spool.tile([128, 1], F32, tag="sumx")
                    nc.vector.tensor_reduce(sumx[:], xt[:], AX.X, OP.add)
                    sqj = sqpool.tile([128, D], F32)
                    ssq = spool.tile([128, 1], F32, tag="ssq")
                    nc.scalar.activation(sqj[:], xt[:], AF.Square, accum_out=ssq[:])
                    mu = spool.tile([128, 1], F32, tag="mu")
                    nc.vector.tensor_scalar(mu[:], sumx[:], 1.0 / D, None, OP.mult)
                    m2 = spool.tile([128, 1], F32, tag="m2")
                    nc.vector.tensor_tensor(m2[:], mu[:], mu[:], OP.mult)
                    var = spool.tile([128, 1], F32, tag="var")
                    nc.vector.tensor_scalar(
                        var[:], ssq[:], 1.0 / D, m2[:, 0:1], OP.mult, OP.subtract
                    )
                    std = spool.tile([128, 1], F32, tag="std")
                    nc.scalar.activation(std[:], var[:], AF.Sqrt, bias=eps_sb[:, 0:1])
                    rstd = spool.tile([128, 1], F32, tag="rstd")
                    nc.vector.reciprocal(rstd[:], std[:])
                    xnp = xnppool.tile([128, D], F32)
                    nc.vector.tensor_scalar(
                        xnp[:], xt[:], mu[:, 0:1], rstd[:, 0:1], OP.subtract, OP.mult
                    )
                    pt = psa.tile([128, 128], F32, space="PSUM")
                    nc.tensor.transpose(pt[:], xnp[:], ident_sb[:])
                    xnpT = xnptpool.tile([128, 128], F32)
                    nc.scalar.copy(xnpT[:], pt[:])
                    pm = psb.tile([128, 129], F32, space="PSUM")
                    nc.tensor.matmul(
                        pm[:], lhsT=xnpT[:], rhs=wext_sb[:], start=True, stop=True
                    )
                    xpe = xpepool.tile([128, 128], BF16)
                    nc.vector.tensor_tensor(
                        xpe[:], pm[:, 0:128], c2b_sb[:, 0:128], OP.add
                    )
                    nc.vector.tensor_tensor(
                        adst_all[:, i : i + 1], pm[:, 128:129], c2b_sb[:, 128:129],
                        OP.add,
                    )
                    nc.sync.dma_start(
                        xp_chunk[i * 128 : i * 128 + rows, :], xpe[:rows, :]
                    )

            nc.gpsimd.collective_compute(
                "AllGather",
                OP.bypass,
                replica_groups=[list(range(NCORES))],
                ins=[xp_chunk[:, :]],
                outs=[xp_full[:, :]],
            )
            if DEBUG_XP:
                with tc.tile_pool(name="dbgxp", bufs=3) as dpool:
                    for i in range(NBLK):
                        rows = 128 if i < NBLK - 1 else LAST_ROWS
                        tb_ = dpool.tile([128, 129], BF16, tag="tb")
                        nc.sync.dma_start(
                            tb_[:rows, :], xp_chunk[i * 128 : i * 128 + rows, 0:129]
                        )
                        tf_ = dpool.tile([128, 128], F32, tag="tf")
                        nc.vector.tensor_copy(tf_[:, 0:127], tb_[:, 0:127])
                        nc.vector.tensor_copy(tf_[:, 127:128], tb_[:, 128:129])
                        nc.sync.dma_start(
                            out_shard[i * 128 : i * 128 + rows, :], tf_[:rows, :]
                        )

            # ---------------- Phase B: edge aggregation --------------------
            with (
                tc.tile_pool(name="b_g", bufs=3) as gpool,
                tc.tile_pool(name="b_oh", bufs=2) as ohpool,
                tc.tile_pool(name="b_ohT", bufs=2) as ohtpool,
                tc.tile_pool(name="b_sw", bufs=2) as swpool,
                tc.tile_pool(name="b_e", bufs=3) as epool,
                tc.tile_pool(name="b_fin", bufs=3) as fpool,
                tc.tile_pool(name="b_psA", bufs=2, space="PSUM") as psA_pool,
                tc.tile_pool(name="b_psB", bufs=GBLK + 1, space="PSUM") as psB_pool,
                tc.tile_pool(name="b_psC", bufs=2, space="PSUM") as psC_pool,
            ):
                psB = {}
                qn = 0
                for gi, (tiles, cb0, nlo) in enumerate([] if DEBUG_XP else groups):
                    T = len(tiles)
                    gf = gpool.tile([128, TG_MAX, ROW], BF16, tag="gf")
                    if nlo:
                        nc.gpsimd.dma_gather(
                            out_ap=gf[:, 0:nlo, :],
                            in_ap=xp_full[0:HALF, :],
                            idxs_ap=fidx_sb[:, cb0 * 8 : (cb0 + nlo) * 8],
                            num_idxs=nlo * 128,
                            num_idxs_reg=nlo * 128,
                            elem_size=ROW,
                            single_packet=False,
                            queue_num=qn % NQ,
                        )
                        qn += 1
                    if T - nlo:
                        nc.gpsimd.dma_gather(
                            out_ap=gf[:, nlo:T, :],
                            in_ap=xp_full[HALF:N, :],
                            idxs_ap=fidx_sb[:, (cb0 + nlo) * 8 : (cb0 + T) * 8],
                            num_idxs=(T - nlo) * 128,
                            num_idxs_reg=(T - nlo) * 128,
                            elem_size=ROW,
                            single_packet=False,
                            queue_num=qn % NQ,
                        )
                        qn += 1
                    oh_sb = ohpool.tile([128, TG_MAX * 128], BF16, tag="oh")
                    nc.sync.dma_start(
                        oh_sb[:, 0 : T * 128],
                        oh_in[:, cb0 * 128 : (cb0 + T) * 128],
                    )
                    ohT_sb = ohtpool.tile([128, TG_MAX * 128], BF16, tag="ohT")
                    nc.sync.dma_start(
                        ohT_sb[:, 0 : T * 128],
                        ohT_in[:, cb0 * 128 : (cb0 + T) * 128],
                    )

                    # a_dst per slot via 1-col matmuls on the transposed one-hot;
                    # cols [TG_MAX:] hold the per-tile denominator columns
                    psA = psA_pool.tile([128, 2 * TG_MAX], F32, space="PSUM")
                    for j, (b, hf) in enumerate(tiles):
                        nc.tensor.matmul(
                            psA[:, j : j + 1],
                            lhsT=ohT_sb[:, j * 128 : (j + 1) * 128],
                            rhs=adst_all[:, b : b + 1],
                            start=True,
                            stop=True,
                        )
                    # a_src per slot: batched multiply + reduce over features
                    prod = gpool.tile([128, TG_MAX, ROW], BF16, tag="prod")
                    nc.vector.tensor_tensor(
                        prod[:, 0:T, :],
                        gf[:, 0:T, :],
                        vsrc_sb.rearrange("p (a b) -> p a b", a=1)[:].to_broadcast(
                            [128, T, ROW]
                        ),
                        OP.mult,
                    )
                    asrcg = epool.tile([128, TG_MAX], F32, tag="asrcg")
                    nc.vector.tensor_reduce(
                        asrcg[:, 0:T], prod[:, 0:T, :], AX.X, OP.add
                    )
                    # ee = exp(leakyrelu(a_src + a_dst))
                    e1 = epool.tile([128, TG_MAX], F32, tag="e1")
                    nc.vector.tensor_tensor(
                        e1[:, 0:T], asrcg[:, 0:T], psA[:, 0:T], OP.add
                    )
                    e2 = epool.tile([128, TG_MAX], F32, tag="e2")
                    nc.vector.tensor_scalar(
                        e2[:, 0:T], e1[:, 0:T], NEG_SLOPE, None, OP.mult
                    )
                    e3 = epool.tile([128, TG_MAX], F32, tag="e3")
                    nc.vector.tensor_tensor(
                        e3[:, 0:T], e2[:, 0:T], e1[:, 0:T], OP.max
                    )
                    ee = epool.tile([128, TG_MAX], F32, tag="ee")
                    if DEBUG_EE1:
                        nc.vector.memset(ee[:], 1.0)
                    else:
                        nc.scalar.activation(ee[:, 0:T], e3[:, 0:T], AF.Exp)

                    eeb = epool.tile([128, TG_MAX], BF16, tag="eeb")
                    nc.vector.tensor_copy(eeb[:, 0:T], ee[:, 0:T])
                    # block-major column position for each tile j in this group
                    order = sorted(range(T), key=lambda j: (tiles[j][0], tiles[j][1]))
                    bm_pos = [0] * T
                    bm_start = {}
                    for pos, j in enumerate(order):
                        bm_pos[j] = pos
                        bm_start.setdefault(tiles[j][0], pos)
                    sw = swpool.tile([128, TG_MAX * 128], BF16, tag="sw")
                    nc.vector.tensor_tensor(
                        sw.rearrange("p (a b) -> p a b", b=128)[:, 0:T, :],
                        oh_sb.rearrange("p (a b) -> p a b", b=128)[:, 0:T, :],
                        ee.rearrange("p (a b) -> p a b", b=1)[:, 0:T, :].to_broadcast(
                            [128, T, 128]
                        ),
                        OP.mult,
                    )
                    if DEBUG_GF:
                        for j, (b, hf) in enumerate(tiles):
                            cb = cb0 + j
                            if first_cb[b] != cb:
                                continue
                            rows = 128 if b < NBLK - 1 else LAST_ROWS
                            gdump = fpool.tile([128, D], F32, tag="outt")
                            nc.vector.tensor_copy(gdump[:], gf[:, j, 0:128])
                            nc.sync.dma_start(
                                out_shard[b * 128 : b * 128 + rows, :], gdump[:rows, :]
                            )
                        continue
                    for j, (b, hf) in enumerate(tiles):
                        cb = cb0 + j
                        if first_cb[b] == cb:
                            psB[b] = psB_pool.tile(
                                [128, 128], F32, space="PSUM", tag="psB", name=f"psB{b}"
                            )
                        nc.tensor.matmul(
                            psB[b][:, :],
                            lhsT=gf[:, j, :],
                            rhs=sw[:, j * 128 : (j + 1) * 128],
                            start=(first_cb[b] == cb),
                            stop=(last_cb[b] == cb),
                        )
                        nc.tensor.matmul(
                            psA[:, TG_MAX + bm_pos[j] : TG_MAX + bm_pos[j] + 1],
                            lhsT=oh_sb[:, j * 128 : (j + 1) * 128],
                            rhs=eeb[:, j : j + 1],
                            start=True,
                            stop=True,
                        )
                        if last_cb[b] != cb:
                            continue
                        # -------- block finale --------
                        rows = 128 if b < NBLK - 1 else LAST_ROWS
                        Sb = fpool.tile([128, 128], BF16, tag="Sb")
                        nc.scalar.copy(Sb[:], psB[b][:, :])
                        psC = psC_pool.tile([128, 128], BF16, space="PSUM")
                        nc.tensor.transpose(psC[:, 0:128], Sb[:], identb_sb[:])
                        nblk_tiles = sum(1 for (b2, _) in tiles if b2 == b)
                        dsum = fpool.tile([128, 1], F32, tag="dsum")
                        nc.vector.tensor_reduce(
                            dsum[:],
                            psA[:, TG_MAX + bm_start[b] : TG_MAX + bm_start[b] + nblk_tiles],
                            AX.X,
                            OP.add,
                        )
                        recip = fpool.tile([128, 1], F32, tag="recip")
                        nc.vector.reciprocal(recip[:], dsum[:])
                        resid = fpool.tile([128, D], F32, tag="resid")
                        nc.vector.scalar_tensor_tensor(
                            resid[:], psC[:, 0:128], recip[:, 0:1], x_tiles[b][:],
                            OP.mult, OP.add,
                        )
                        outt = fpool.tile([128, D], F32, tag="outt")
                        if DEBUG_RAW:
                            nc.vector.tensor_copy(outt[:, 0:1], psC[:, 128:129])
                            nc.vector.tensor_copy(outt[:, 1:128], psC[:, 1:128])
                        else:
                            nc.scalar.activation(outt[:], resid[:], AF.Relu)
                        nc.sync.dma_start(
                            out_shard[b * 128 : b * 128 + rows, :], outt[:rows, :]
                        )

    nc.compile()
    return nc


def _wrap_idx(idx):
    """int16 index list -> dma_gather SBUF layout [128, len/16]."""
    L = len(idx)
    assert L % 16 == 0
    w = idx.reshape(L // 16, 16).T.astype(np.int16)
    return np.tile(w, (8, 1))


def _host_prep(x, edge_index, ln_gamma, ln_beta, W, att_src, att_dst, bias):
    """Fold parameters, bucket edges, build one-hot tables. Numpy only."""
    Wt = W.T.astype(np.float64)
    G = ln_gamma.astype(np.float64)[:, None] * Wt          # [D, D]
    crow = ln_beta.astype(np.float64) @ Wt                 # [D]
    v_src = G @ att_src.astype(np.float64)
    v_dst = G @ att_dst.astype(np.float64)
    c_src = float(crow @ att_src.astype(np.float64))
    c_dst = float(crow @ att_dst.astype(np.float64))

    wext = np.zeros((D, 129), np.float32)
    wext[:, 0:D] = G.astype(np.float32)
    wext[:, 128] = v_dst.astype(np.float32)
    c2 = np.zeros((129,), np.float32)
    c2[0:D] = (crow + bias.astype(np.float64)).astype(np.float32)
    # table rows hold xp+bias; their dot with att_src carries an extra
    # bias@att_src which we cancel inside the (src+dst)-additive constant
    c2[128] = np.float32(
        c_dst - float(bias.astype(np.float64) @ att_src.astype(np.float64))
    )
    c2b = np.broadcast_to(c2, (128, 129)).copy()
    ident = np.eye(128, dtype=np.float32)
    vsrcb = np.broadcast_to(
        att_src.astype(np.float32), (128, D)
    ).astype(ml_dtypes.bfloat16).copy()

    # edges + self loops, sorted by (core, block, src-half)
    src = np.concatenate([edge_index[0], np.arange(N, dtype=np.int64)]).astype(np.int64)
    dst = np.concatenate([edge_index[1], np.arange(N, dtype=np.int64)]).astype(np.int64)
    core = dst // SHARD
    local = dst - core * SHARD
    blk = local // 128
    half = (src >= HALF).astype(np.int64)
    key = (core * NBLK + blk) * 2 + half
    order = np.argsort(key, kind="stable")
    src, dst, key = src[order], dst[order], key[order]
    counts = np.bincount(key, minlength=NCORES * NBLK * 2).reshape(NCORES, NBLK, 2)
    tiles = -(-counts // 128)                              # ceil
    tlo = tuple(int(t) for t in tiles[:, :, 0].max(axis=0))
    thi = tuple(int(t) for t in tiles[:, :, 1].max(axis=0))
    CB = sum(tlo) + sum(thi)

    # slot offset (in tiles) of each (block, half) segment, same for all cores
    seg_off = {}
    cb0 = 0
    for g0 in range(0, NBLK, GBLK):
        blocks = list(range(g0, min(NBLK, g0 + GBLK)))
        off = cb0
        for b in blocks:
            seg_off[(b, 0)] = off
            off += tlo[b]
        for b in blocks:
            seg_off[(b, 1)] = off
            off += thi[b]
        cb0 = off
    assert cb0 == CB

    starts = np.zeros(NCORES * NBLK * 2 + 1, np.int64)
    starts[1:] = np.cumsum(counts.reshape(-1))

    feat_idx = np.zeros((NCORES, CB * 128), np.int16)
    oh = np.zeros((NCORES, 128, CB * 128), ml_dtypes.bfloat16)
    ohT = np.zeros((NCORES, 128, CB * 128), ml_dtypes.bfloat16)

    for c in range(NCORES):
        for b in range(NBLK):
            for hf in range(2):
                gidx = (c * NBLK + b) * 2 + hf
                s, e = starts[gidx], starts[gidx + 1]
                n = int(e - s)
                if n == 0:
                    continue
                off = seg_off[(b, hf)]
                k = np.arange(n) + off * 128           # global slot ids
                fi = (src[s:e] - hf * HALF).astype(np.int16)
                feat_idx[c, k] = fi
                dl = (dst[s:e] - (c * SHARD + b * 128)).astype(np.int64)
                p = k % 128
                t = k // 128
                oh[c, p, t * 128 + dl] = 1
                ohT[c, dl, t * 128 + p] = 1

    in_maps = []
    for c in range(NCORES):
        xs = np.zeros((PAD_SHARD, D), np.float32)
        xs[0:SHARD] = x[c * SHARD : (c + 1) * SHARD]
        in_maps.append(
            {
                "x_shard": xs,
                "wext": wext,
                "c2b": c2b,
                "vsrcb": vsrcb,
                "ident": ident,
                "feat_idx": _wrap_idx(feat_idx[c]),
                "oh_in": np.ascontiguousarray(oh[c]),
                "ohT_in": np.ascontiguousarray(ohT[c]),
            }
        )
    return tlo, thi, in_maps


_PROGRAM_CACHE = {}


def kernel(x, edge_index, edge_attr, h, batch, ln_gamma, ln_beta, W, att_src,
           att_dst, bias):
    x = np.asarray(x, dtype=np.float32)
    edge_index = np.asarray(edge_index)
    h = np.asarray(h)
    ln_gamma = np.asarray(ln_gamma, dtype=np.float32)
    ln_beta = np.asarray(ln_beta, dtype=np.float32)
    W = np.asarray(W, dtype=np.float32)
    att_src = np.asarray(att_src, dtype=np.float32)
    att_dst = np.asarray(att_dst, dtype=np.float32)
    bias = np.asarray(bias, dtype=np.float32)

    tlo, thi, in_maps = _host_prep(
        x, edge_index, ln_gamma, ln_beta, W, att_src, att_dst, bias
    )
    key = (tlo, thi)
    if key not in _PROGRAM_CACHE:
        _PROGRAM_CACHE[key] = _build_program(tlo, thi)
    nc = _PROGRAM_CACHE[key]

    res = run_bass_kernel_spmd(nc, in_maps, core_ids=list(range(NCORES)))
    out = np.concatenate([res.results[c]["out_shard"] for c in range(NCORES)], axis=0)
    return out, h



# revision 12
# speedup vs baseline: 1.0384x; 1.0384x over previous
"""GAT layer (LayerNorm -> GATConv(heads=1) -> residual ReLU) on 8 trn2 NeuronCores.

Sharding: destination-node parallel. Each core owns N/8 nodes, computes the
fused LN+linear transform for its shard (bf16 rows of 128 feats at 256 B
pitch plus an a_dst column per dst block), AllGathers the bf16 node table,
then processes the edges whose destination falls in its shard.

Edges are bucketed by (dst block, src half) into 128-slot tiles. Per tile:
source rows arrive via dma_gather (256 B elems, SWDGE queues round-robin),
a_dst is broadcast to slots with a 1-column matmul against a host-shipped
transposed one-hot (ohT), e = a_src + a_dst comes from one fused
tensor_tensor_reduce per tile (dot(row, att_src) with the a_dst column as
the reduction seed), ee = exp(lrelu(e)) on ACT, sw = (iota == dl) * ee is
built on-device (no dense one-hot from HBM), and one matmul per tile
accumulates the dst-major scatter psBT[dst, feat] += sw^T-contract-gf with
a second 1-column matmul giving the softmax denominator in psum column 128.
Block finale: normalize, add residual, ReLU - all on DVE so the ACT queue
stays short.
"""

import numpy as np
import ml_dtypes

import concourse.bacc as bacc
import concourse.mybir as mybir
import concourse.tile as tile
from concourse.bass_utils import run_bass_kernel_spmd

F32 = mybir.dt.float32
BF16 = mybir.dt.bfloat16
I16 = mybir.dt.int16
I32 = mybir.dt.int32
AX = mybir.AxisListType
OP = mybir.AluOpType
AF = mybir.ActivationFunctionType

N = 50000
D = 128
E = 600000
NCORES = 8
SHARD = N // NCORES            # 6250
NBLK = (SHARD + 127) // 128    # 49 dst blocks per core
PAD_SHARD = NBLK * 128         # 6272
LAST_ROWS = SHARD - (NBLK - 1) * 128  # 106
ROW = 128                      # bf16 cols per table row (256 B pitch)
HALF = 32768                   # int16 index split point for the global table
NEG_SLOPE = 0.2
LN_EPS = 1e-5
GBLK = 3                       # dst blocks per group
import os as _os
NQ = int(_os.environ.get("KNQ", "4"))


def _build_program(tlo, thi):
    """One SPMD program; per-core behaviour differs only through its inputs."""
    nc = bacc.Bacc("TRN2", num_devices=NCORES, debug=False, num_swdge_queues=NQ)

    CB = sum(tlo) + sum(thi)   # total column-block tiles per core

    x_shard = nc.dram_tensor("x_shard", [PAD_SHARD, D], F32, kind="ExternalInput")
    wext = nc.dram_tensor("wext", [D, 129], F32, kind="ExternalInput")
    c2b = nc.dram_tensor("c2b", [128, 129], F32, kind="ExternalInput")
    vsrcb = nc.dram_tensor("vsrcb", [128, 128], BF16, kind="ExternalInput")
    ident = nc.dram_tensor("ident", [128, 128], F32, kind="ExternalInput")
    feat_idx = nc.dram_tensor("feat_idx", [128, CB * 8], I16, kind="ExternalInput")
    ohT_in = nc.dram_tensor("ohT_in", [128, CB * 128], BF16, kind="ExternalInput")
    dl_in = nc.dram_tensor("dl_in", [128, CB], F32, kind="ExternalInput")
    iota_in = nc.dram_tensor("iota_in", [128, 128], F32, kind="ExternalInput")
    out_shard = nc.dram_tensor("out_shard", [PAD_SHARD, D], F32, kind="ExternalOutput")

    # group structure: per group, lo tiles of its blocks then hi tiles
    groups = []  # (tiles, cb0, nlo) ; tiles = list of (block, half)
    cb0 = 0
    for g0 in range(0, NBLK, GBLK):
        blocks = list(range(g0, min(NBLK, g0 + GBLK)))
        tiles = []
        for b in blocks:
            tiles += [(b, 0)] * tlo[b]
        nlo = len(tiles)
        for b in blocks:
            tiles += [(b, 1)] * thi[b]
        groups.append((tiles, cb0, nlo))
        cb0 += len(tiles)
    assert cb0 == CB
    TG_MAX = max(len(t) for t, _, _ in groups)

    # first/last tile index (within CB) per block, for psum start/stop
    first_cb = {}
    last_cb = {}
    for tiles, c0, _ in groups:
        for j, (b, hf) in enumerate(tiles):
            cb = c0 + j
            first_cb.setdefault(b, cb)
            last_cb[b] = cb

    with tile.TileContext(nc) as tc:
        with (
            tc.tile_pool(name="dram", bufs=1, space="DRAM") as dram,
            tc.tile_pool(name="consts", bufs=1) as cpool,
            tc.tile_pool(name="xres", bufs=1) as xpool,
        ):
            xp_chunk = dram.tile([PAD_SHARD, ROW], BF16)
            xp_full = dram.tile([N, ROW], BF16, addr_space="Shared")

            ident_sb = cpool.tile([128, 128], F32)
            nc.sync.dma_start(ident_sb[:], ident[:, :])
            wext_sb = cpool.tile([D, 129], F32)
            nc.sync.dma_start(wext_sb[:], wext[:, :])
            c2b_sb = cpool.tile([128, 129], F32)
            nc.sync.dma_start(c2b_sb[:], c2b[:, :])
            vsrc_sb = cpool.tile([128, 128], BF16)
            nc.sync.dma_start(vsrc_sb[:], vsrcb[:, :])
            eps_sb = cpool.tile([128, 1], F32)
            nc.vector.memset(eps_sb[:], LN_EPS)
            ones_sb = cpool.tile([128, 1], BF16)
            nc.vector.memset(ones_sb[:], 1.0)
            fidx_sb = cpool.tile([128, CB * 8], I16)
            nc.sync.dma_start(fidx_sb[:], feat_idx[:, :])
            dl_sb = cpool.tile([128, CB], F32)
            nc.sync.dma_start(dl_sb[:], dl_in[:, :])
            iota_sb = cpool.tile([128, 128], F32)
            nc.sync.dma_start(iota_sb[:], iota_in[:, :])
            adst_all = cpool.tile([128, NBLK], BF16)

            # residual x for the whole shard, one DMA
            xres = xpool.tile([128, NBLK, D], F32)
            nc.sync.dma_start(
                xres[:], x_shard.rearrange("(b p) d -> p b d", p=128)[:]
            )
            # bf16 node-table staging for the whole shard
            xpstage = xpool.tile([128, NBLK, ROW], BF16)

            # ---------------- Phase A: node transform on own shard ---------
            with (
                tc.tile_pool(name="a_small", bufs=2) as spool,
                tc.tile_pool(name="a_sq", bufs=2) as sqpool,
                tc.tile_pool(name="a_xnp", bufs=3) as xnppool,
                tc.tile_pool(name="a_xnpT", bufs=3) as xnptpool,
                tc.tile_pool(name="a_ps_t", bufs=2, space="PSUM") as psa,
                tc.tile_pool(name="a_ps_m", bufs=2, space="PSUM") as psb,
            ):
                sumx = spool.tile([128, NBLK], F32, tag="sumx")
                nc.vector.tensor_reduce(sumx[:], xres[:], AX.X, OP.add)
                ssq = spool.tile([128, NBLK], F32, tag="ssq")
                for i in range(NBLK):
                    sqj = sqpool.tile([128, D], F32)
                    nc.scalar.activation(
                        sqj[:], xres[:, i, :], AF.Square, accum_out=ssq[:, i : i + 1]
                    )
                mu = spool.tile([128, NBLK], F32, tag="mu")
                nc.vector.tensor_scalar(mu[:], sumx[:], 1.0 / D, None, OP.mult)
                m2 = spool.tile([128, NBLK], F32, tag="m2")
                nc.vector.tensor_tensor(m2[:], mu[:], mu[:], OP.mult)
                var = spool.tile([128, NBLK], F32, tag="var")
                nc.vector.scalar_tensor_tensor(
                    var[:], ssq[:], 1.0 / D, m2[:], OP.mult, OP.subtract
                )
                std = spool.tile([128, NBLK], F32, tag="std")
                nc.scalar.activation(std[:], var[:], AF.Sqrt, bias=eps_sb[:, 0:1])
                rstd = spool.tile([128, NBLK], F32, tag="rstd")
                nc.vector.reciprocal(rstd[:], std[:])

                for i in range(NBLK):
                    xnp = xnppool.tile([128, D], F32)
                    nc.vector.tensor_scalar(
                        xnp[:], xres[:, i, :], mu[:, i : i + 1], rstd[:, i : i + 1],
                        OP.subtract, OP.mult,
                    )
                    pt = psa.tile([128, 128], F32, space="PSUM")
                    nc.tensor.transpose(pt[:], xnp[:], ident_sb[:])
                    xnpT = xnptpool.tile([128, 128], F32)
                    nc.scalar.copy(xnpT[:], pt[:])
                    pm = psb.tile([128, 129], F32, space="PSUM")
                    nc.tensor.matmul(
                        pm[:], lhsT=xnpT[:], rhs=wext_sb[:], start=True, stop=True
                    )
                    nc.vector.tensor_tensor(
                        xpstage[:, i, :], pm[:, 0:128], c2b_sb[:, 0:128], OP.add
                    )
                    nc.vector.tensor_tensor(
                        adst_all[:, i : i + 1], pm[:, 128:129], c2b_sb[:, 128:129],
                        OP.add,
                    )

            nc.sync.dma_start(
                xp_chunk.rearrange("(b p) d -> p b d", p=128)[:],
                xpstage[:],
            )

            nc.gpsimd.collective_compute(
                "AllGather",
                OP.bypass,
                replica_groups=[list(range(NCORES))],
                ins=[xp_chunk[0:SHARD, :]],
                outs=[xp_full[:, :]],
            )

            # ---------------- Phase B: edge aggregation --------------------
            with (
                tc.tile_pool(name="b_g", bufs=3) as gpool,
                tc.tile_pool(name="b_ohT", bufs=2) as ohtpool,
                tc.tile_pool(name="b_sw", bufs=2) as swpool,
                tc.tile_pool(name="b_e", bufs=3) as epool,
                tc.tile_pool(name="b_sc", bufs=2) as scpool,
                tc.tile_pool(name="b_fin", bufs=2) as fpool,
                tc.tile_pool(name="b_out", bufs=2) as opool,
                tc.tile_pool(name="b_psA", bufs=2, space="PSUM") as psA_pool,
                tc.tile_pool(name="b_psB", bufs=GBLK + 1, space="PSUM") as psB_pool,
            ):
                psBT = {}
                qn = 0
                for gi, (tiles, cb0, nlo) in enumerate(groups):
                    T = len(tiles)
                    g0 = tiles[0][0]
                    nb = tiles[-1][0] - g0 + 1   # blocks in this group
                    gf = gpool.tile([128, TG_MAX, ROW], BF16, tag="gf")
                    if nlo:
                        nc.gpsimd.dma_gather(
                            out_ap=gf[:, 0:nlo, :],
                            in_ap=xp_full[0:HALF, :],
                            idxs_ap=fidx_sb[:, cb0 * 8 : (cb0 + nlo) * 8],
                            num_idxs=nlo * 128,
                            num_idxs_reg=nlo * 128,
                            elem_size=ROW,
                            single_packet=False,
                            queue_num=qn % NQ,
                        )
                        qn += 1
                    if T - nlo:
                        nc.gpsimd.dma_gather(
                            out_ap=gf[:, nlo:T, :],
                            in_ap=xp_full[HALF:N, :],
                            idxs_ap=fidx_sb[:, (cb0 + nlo) * 8 : (cb0 + T) * 8],
                            num_idxs=(T - nlo) * 128,
                            num_idxs_reg=(T - nlo) * 128,
                            elem_size=ROW,
                            single_packet=False,
                            queue_num=qn % NQ,
                        )
                        qn += 1
                    ohT_sb = ohtpool.tile([128, TG_MAX * 128], BF16, tag="ohT")
                    nc.sync.dma_start(
                        ohT_sb[:, 0 : T * 128],
                        ohT_in[:, cb0 * 128 : (cb0 + T) * 128],
                    )

                    # block-major column position for each tile j in this group
                    order = sorted(range(T), key=lambda j: (tiles[j][0], tiles[j][1]))
                    bm_pos = [0] * T
                    bm_start = {}
                    for pos, j in enumerate(order):
                        bm_pos[j] = pos
                        bm_start.setdefault(tiles[j][0], pos)
                    # a_dst per slot via 1-col matmuls on the transposed one-hot;
                    # cols [TG_MAX:] hold the per-tile denominator columns
                    psA = psA_pool.tile([128, 2 * TG_MAX], F32, space="PSUM")
                    for j, (b, hf) in enumerate(tiles):
                        nc.tensor.matmul(
                            psA[:, j : j + 1],
                            lhsT=ohT_sb[:, j * 128 : (j + 1) * 128],
                            rhs=adst_all[:, b : b + 1],
                            start=True,
                            stop=True,
                        )
                    # a_src per slot: fused dot(row, att_src) via stt accum
                    asrcg = epool.tile([128, TG_MAX], F32, tag="asrcg")
                    for j in range(T):
                        scr = scpool.tile([128, ROW], BF16, tag="scr")
                        nc.vector.scalar_tensor_tensor(
                            scr[:], gf[:, j, :], 1.0, vsrc_sb[:],
                            OP.mult, OP.mult,
                            accum_out=asrcg[:, j : j + 1],
                        )
                    e1 = epool.tile([128, TG_MAX], F32, tag="e1")
                    nc.vector.tensor_tensor(
                        e1[:, 0:T], asrcg[:, 0:T], psA[:, 0:T], OP.add
                    )
                    # ee = exp(leakyrelu(e1)); lrelu as (e1*slope) max e1 on DVE
                    e3 = epool.tile([128, TG_MAX], F32, tag="e3")
                    nc.vector.scalar_tensor_tensor(
                        e3[:, 0:T], e1[:, 0:T], NEG_SLOPE, e1[:, 0:T],
                        OP.mult, OP.max,
                    )
                    ee = epool.tile([128, TG_MAX], F32, tag="ee")
                    nc.scalar.activation(ee[:, 0:T], e3[:, 0:T], AF.Exp)

                    # sw[slot, d] = (iota == dl) * ee  (on-device one-hot)
                    sw = swpool.tile([128, TG_MAX, 128], BF16, tag="sw")
                    for j in range(T):
                        nc.vector.tensor_scalar(
                            sw[:, j, :], iota_sb[:],
                            dl_sb[:, cb0 + j : cb0 + j + 1],
                            ee[:, j : j + 1],
                            OP.is_equal, OP.mult,
                        )
                    outg = opool.tile([128, GBLK, D], F32, tag="outg")
                    for j, (b, hf) in enumerate(tiles):
                        cb = cb0 + j
                        if first_cb[b] == cb:
                            psBT[b] = psB_pool.tile(
                                [128, 128], F32, space="PSUM", tag="psBT",
                                name=f"psBT{b}",
                            )
                        nc.tensor.matmul(
                            psBT[b][:, :],
                            lhsT=sw[:, j, :],
                            rhs=gf[:, j, :],
                            start=(first_cb[b] == cb),
                            stop=(last_cb[b] == cb),
                        )
                        nc.tensor.matmul(
                            psA[:, TG_MAX + bm_pos[j] : TG_MAX + bm_pos[j] + 1],
                            lhsT=sw[:, j, :],
                            rhs=ones_sb[:],
                            start=True,
                            stop=True,
                        )
                        if last_cb[b] != cb:
                            continue
                        # -------- block finale (all on DVE) --------
                        nblk_tiles = sum(1 for (b2, _) in tiles if b2 == b)
                        dsum = fpool.tile([128, 1], F32, tag="dsum")
                        nc.vector.tensor_reduce(
                            dsum[:],
                            psA[:, TG_MAX + bm_start[b] : TG_MAX + bm_start[b] + nblk_tiles],
                            AX.X,
                            OP.add,
                        )
                        recip = fpool.tile([128, 1], F32, tag="recip")
                        nc.vector.reciprocal(recip[:], dsum[:])
                        outt = fpool.tile([128, D], F32, tag="outt")
                        nc.vector.scalar_tensor_tensor(
                            outt[:], psBT[b][:, :], recip[:, 0:1],
                            xres[:, b, :], OP.mult, OP.add,
                        )
                        nc.vector.tensor_scalar(
                            outg[:, b - g0, :], outt[:], 0.0, None, OP.max
                        )
                    nc.sync.dma_start(
                        out_shard.rearrange("(b p) d -> p b d", p=128)[
                            :, g0 : g0 + nb, :
                        ],
                        outg[:, 0:nb, :],
                    )

    nc.compile()
    return nc


def _wrap_idx(idx):
    """int16 index list -> dma_gather SBUF layout [128, len/16]."""
    L = len(idx)
    assert L % 16 == 0
    w = idx.reshape(L // 16, 16).T.astype(np.int16)
    return np.tile(w, (8, 1))


def _host_prep(x, edge_index, ln_gamma, ln_beta, W, att_src, att_dst, bias):
    """Fold parameters, bucket edges, build index tables. Numpy only."""
    Wt = W.T.astype(np.float64)
    G = ln_gamma.astype(np.float64)[:, None] * Wt          # [D, D]
    crow = ln_beta.astype(np.float64) @ Wt                 # [D]
    v_dst = G @ att_dst.astype(np.float64)
    c_dst = float(crow @ att_dst.astype(np.float64))

    wext = np.zeros((D, 129), np.float32)
    wext[:, 0:D] = G.astype(np.float32)
    wext[:, 128] = v_dst.astype(np.float32)
    c2 = np.zeros((129,), np.float32)
    c2[0:D] = (crow + bias.astype(np.float64)).astype(np.float32)
    # table rows hold xp+bias; their dot with att_src carries an extra
    # bias@att_src which we cancel inside the (src+dst)-additive constant
    c2[128] = np.float32(
        c_dst - float(bias.astype(np.float64) @ att_src.astype(np.float64))
    )
    c2b = np.broadcast_to(c2, (128, 129)).copy()
    ident = np.eye(128, dtype=np.float32)
    vsrcb = np.broadcast_to(
        att_src.astype(np.float32), (128, D)
    ).astype(ml_dtypes.bfloat16).copy()

    # edges + self loops, sorted by (core, block, src-half)
    src = np.concatenate([edge_index[0], np.arange(N, dtype=np.int64)]).astype(np.int64)
    dst = np.concatenate([edge_index[1], np.arange(N, dtype=np.int64)]).astype(np.int64)
    core = dst // SHARD
    local = dst - core * SHARD
    blk = local // 128
    half = (src >= HALF).astype(np.int64)
    key = (core * NBLK + blk) * 2 + half
    order = np.argsort(key, kind="stable")
    src, dst, key = src[order], dst[order], key[order]
    counts = np.bincount(key, minlength=NCORES * NBLK * 2).reshape(NCORES, NBLK, 2)
    tiles = -(-counts // 128)                              # ceil
    tlo = tuple(int(t) for t in tiles[:, :, 0].max(axis=0))
    thi = tuple(int(t) for t in tiles[:, :, 1].max(axis=0))
    CB = sum(tlo) + sum(thi)

    # slot offset (in tiles) of each (block, half) segment, same for all cores
    seg_off = {}
    cb0 = 0
    for g0 in range(0, NBLK, GBLK):
        blocks = list(range(g0, min(NBLK, g0 + GBLK)))
        off = cb0
        for b in blocks:
            seg_off[(b, 0)] = off
            off += tlo[b]
        for b in blocks:
            seg_off[(b, 1)] = off
            off += thi[b]
        cb0 = off
    assert cb0 == CB

    starts = np.zeros(NCORES * NBLK * 2 + 1, np.int64)
    starts[1:] = np.cumsum(counts.reshape(-1))

    feat_idx = np.zeros((NCORES, CB * 128), np.int16)
    ohT = np.zeros((NCORES, 128, CB * 128), ml_dtypes.bfloat16)
    dl_all = np.full((NCORES, 128, CB), -1.0, np.float32)

    for c in range(NCORES):
        for b in range(NBLK):
            for hf in range(2):
                gidx = (c * NBLK + b) * 2 + hf
                s, e = starts[gidx], starts[gidx + 1]
                n = int(e - s)
                if n == 0:
                    continue
                off = seg_off[(b, hf)]
                k = np.arange(n) + off * 128           # global slot ids
                fi = (src[s:e] - hf * HALF).astype(np.int16)
                feat_idx[c, k] = fi
                dl = (dst[s:e] - (c * SHARD + b * 128)).astype(np.int64)
                p = k % 128
                t = k // 128
                ohT[c, dl, t * 128 + p] = 1
                dl_all[c, p, t] = dl.astype(np.float32)

    in_maps = []
    for c in range(NCORES):
        xs = np.zeros((PAD_SHARD, D), np.float32)
        xs[0:SHARD] = x[c * SHARD : (c + 1) * SHARD]
        in_maps.append(
            {
                "x_shard": xs,
                "wext": wext,
                "c2b": c2b,
                "vsrcb": vsrcb,
                "ident": ident,
                "feat_idx": _wrap_idx(feat_idx[c]),
                "ohT_in": np.ascontiguousarray(ohT[c]),
                "dl_in": np.ascontiguousarray(dl_all[c]),
                "iota_in": np.broadcast_to(
                    np.arange(128, dtype=np.float32), (128, 128)
                ).copy(),
            }
        )
    return tlo, thi, in_maps


_PROGRAM_CACHE = {}


def kernel(x, edge_index, edge_attr, h, batch, ln_gamma, ln_beta, W, att_src,
           att_dst, bias):
    x = np.asarray(x, dtype=np.float32)
    edge_index = np.asarray(edge_index)
    h = np.asarray(h)
    ln_gamma = np.asarray(ln_gamma, dtype=np.float32)
    ln_beta = np.asarray(ln_beta, dtype=np.float32)
    W = np.asarray(W, dtype=np.float32)
    att_src = np.asarray(att_src, dtype=np.float32)
    att_dst = np.asarray(att_dst, dtype=np.float32)
    bias = np.asarray(bias, dtype=np.float32)

    tlo, thi, in_maps = _host_prep(
        x, edge_index, ln_gamma, ln_beta, W, att_src, att_dst, bias
    )
    key = (tlo, thi)
    if key not in _PROGRAM_CACHE:
        _PROGRAM_CACHE[key] = _build_program(tlo, thi)
    nc = _PROGRAM_CACHE[key]

    res = run_bass_kernel_spmd(nc, in_maps, core_ids=list(range(NCORES)))
    out = np.concatenate(
        [res.results[c]["out_shard"][:SHARD] for c in range(NCORES)], axis=0
    )
    return out, h


# revision 16
# speedup vs baseline: 1.0579x; 1.0188x over previous
"""GAT layer (LayerNorm -> GATConv(heads=1) -> residual ReLU) on 8 trn2 NeuronCores.

Sharding: destination-node parallel. Each core owns N/8 nodes, computes the
fused LN+linear transform for its shard (bf16 rows of 128 feats at 256 B
pitch plus an a_dst column per dst block), AllGathers the bf16 node table,
then processes the edges whose destination falls in its shard.

Edges are bucketed by (dst block, src half) into 128-slot tiles. Per tile:
source rows arrive via dma_gather (256 B elems, SWDGE queues round-robin),
a_dst is broadcast to slots with a 1-column matmul against a host-shipped
transposed one-hot (ohT), e = a_src + a_dst comes from one fused
tensor_tensor_reduce per tile (dot(row, att_src) with the a_dst column as
the reduction seed), ee = exp(lrelu(e)) on ACT, sw = (iota == dl) * ee is
built on-device (no dense one-hot from HBM), and one matmul per tile
accumulates the dst-major scatter psBT[dst, feat] += sw^T-contract-gf with
a second 1-column matmul giving the softmax denominator in psum column 128.
Block finale: normalize, add residual, ReLU - all on DVE so the ACT queue
stays short.
"""

import numpy as np
import ml_dtypes

import concourse.bacc as bacc
import concourse.mybir as mybir
import concourse.tile as tile
from concourse.bass_utils import run_bass_kernel_spmd

F32 = mybir.dt.float32
BF16 = mybir.dt.bfloat16
I16 = mybir.dt.int16
I32 = mybir.dt.int32
AX = mybir.AxisListType
OP = mybir.AluOpType
AF = mybir.ActivationFunctionType

N = 50000
D = 128
E = 600000
NCORES = 8
SHARD = N // NCORES            # 6250
NBLK = (SHARD + 127) // 128    # 49 dst blocks per core
PAD_SHARD = NBLK * 128         # 6272
LAST_ROWS = SHARD - (NBLK - 1) * 128  # 106
ROW = 128                      # bf16 cols per table row (256 B pitch)
HALF = 32768                   # int16 index split point for the global table
NEG_SLOPE = 0.2
LN_EPS = 1e-5
GBLK = 3                       # dst blocks per group
import os as _os
NQ = int(_os.environ.get("KNQ", "4"))
SP = _os.environ.get("KSP", "0") == "1"            # single_packet for dma_gather
DEBUG_NOASRC = _os.environ.get("DEBUG_NOASRC") == "1"  # timing probe: skip a_src


def _build_program(tlo, thi):
    """One SPMD program; per-core behaviour differs only through its inputs."""
    nc = bacc.Bacc("TRN2", num_devices=NCORES, debug=False, num_swdge_queues=NQ)

    CB = sum(tlo) + sum(thi)   # total column-block tiles per core

    x_shard = nc.dram_tensor("x_shard", [PAD_SHARD, D], F32, kind="ExternalInput")
    wext = nc.dram_tensor("wext", [D, 129], F32, kind="ExternalInput")
    c2b = nc.dram_tensor("c2b", [128, 129], F32, kind="ExternalInput")
    vsrcb = nc.dram_tensor("vsrcb", [128, 128], BF16, kind="ExternalInput")
    ident = nc.dram_tensor("ident", [128, 128], F32, kind="ExternalInput")
    feat_idx = nc.dram_tensor("feat_idx", [128, CB * 8], I16, kind="ExternalInput")
    ohT_in = nc.dram_tensor("ohT_in", [128, CB * 128], BF16, kind="ExternalInput")
    dl_in = nc.dram_tensor("dl_in", [128, CB], F32, kind="ExternalInput")
    iota_in = nc.dram_tensor("iota_in", [128, 128], F32, kind="ExternalInput")
    out_shard = nc.dram_tensor("out_shard", [PAD_SHARD, D], F32, kind="ExternalOutput")

    # group structure: per group, lo tiles of its blocks then hi tiles
    groups = []  # (tiles, cb0, nlo) ; tiles = list of (block, half)
    cb0 = 0
    for g0 in range(0, NBLK, GBLK):
        blocks = list(range(g0, min(NBLK, g0 + GBLK)))
        tiles = []
        for b in blocks:
            tiles += [(b, 0)] * tlo[b]
        nlo = len(tiles)
        for b in blocks:
            tiles += [(b, 1)] * thi[b]
        groups.append((tiles, cb0, nlo))
        cb0 += len(tiles)
    assert cb0 == CB
    TG_MAX = max(len(t) for t, _, _ in groups)

    # first/last tile index (within CB) per block, for psum start/stop
    first_cb = {}
    last_cb = {}
    for tiles, c0, _ in groups:
        for j, (b, hf) in enumerate(tiles):
            cb = c0 + j
            first_cb.setdefault(b, cb)
            last_cb[b] = cb

    with tile.TileContext(nc) as tc:
        with (
            tc.tile_pool(name="dram", bufs=1, space="DRAM") as dram,
            tc.tile_pool(name="consts", bufs=1) as cpool,
            tc.tile_pool(name="xres", bufs=1) as xpool,
        ):
            xp_chunk = dram.tile([PAD_SHARD, ROW], BF16)
            xp_full = dram.tile([N, ROW], BF16, addr_space="Shared")

            ident_sb = cpool.tile([128, 128], F32)
            nc.sync.dma_start(ident_sb[:], ident[:, :])
            wext_sb = cpool.tile([D, 129], F32)
            nc.sync.dma_start(wext_sb[:], wext[:, :])
            c2b_sb = cpool.tile([128, 129], F32)
            nc.sync.dma_start(c2b_sb[:], c2b[:, :])
            vsrc_sb = cpool.tile([128, 128], BF16)
            nc.sync.dma_start(vsrc_sb[:], vsrcb[:, :])
            eps_sb = cpool.tile([128, 1], F32)
            nc.vector.memset(eps_sb[:], LN_EPS)
            ones_sb = cpool.tile([128, 1], BF16)
            nc.vector.memset(ones_sb[:], 1.0)
            fidx_sb = cpool.tile([128, CB * 8], I16)
            nc.sync.dma_start(fidx_sb[:], feat_idx[:, :])
            dl_sb = cpool.tile([128, CB], F32)
            nc.sync.dma_start(dl_sb[:], dl_in[:, :])
            iota_sb = cpool.tile([128, 128], F32)
            nc.sync.dma_start(iota_sb[:], iota_in[:, :])
            adst_all = cpool.tile([128, NBLK], BF16)

            # residual x for the whole shard, one DMA
            xres = xpool.tile([128, NBLK, D], F32)
            nc.sync.dma_start(
                xres[:], x_shard.rearrange("(b p) d -> p b d", p=128)[:]
            )
            # bf16 node-table staging for the whole shard
            xpstage = xpool.tile([128, NBLK, ROW], BF16)

            # ---------------- Phase A: node transform on own shard ---------
            with (
                tc.tile_pool(name="a_small", bufs=2) as spool,
                tc.tile_pool(name="a_sq", bufs=2) as sqpool,
                tc.tile_pool(name="a_xnp", bufs=3) as xnppool,
                tc.tile_pool(name="a_xnpT", bufs=3) as xnptpool,
                tc.tile_pool(name="a_ps_t", bufs=2, space="PSUM") as psa,
                tc.tile_pool(name="a_ps_m", bufs=2, space="PSUM") as psb,
            ):
                sumx = spool.tile([128, NBLK], F32, tag="sumx")
                nc.vector.tensor_reduce(sumx[:], xres[:], AX.X, OP.add)
                ssq = spool.tile([128, NBLK], F32, tag="ssq")
                for i in range(NBLK):
                    sqj = sqpool.tile([128, D], F32)
                    nc.scalar.activation(
                        sqj[:], xres[:, i, :], AF.Square, accum_out=ssq[:, i : i + 1]
                    )
                mu = spool.tile([128, NBLK], F32, tag="mu")
                nc.vector.tensor_scalar(mu[:], sumx[:], 1.0 / D, None, OP.mult)
                m2 = spool.tile([128, NBLK], F32, tag="m2")
                nc.vector.tensor_tensor(m2[:], mu[:], mu[:], OP.mult)
                var = spool.tile([128, NBLK], F32, tag="var")
                nc.vector.scalar_tensor_tensor(
                    var[:], ssq[:], 1.0 / D, m2[:], OP.mult, OP.subtract
                )
                std = spool.tile([128, NBLK], F32, tag="std")
                nc.scalar.activation(std[:], var[:], AF.Sqrt, bias=eps_sb[:, 0:1])
                rstd = spool.tile([128, NBLK], F32, tag="rstd")
                nc.vector.reciprocal(rstd[:], std[:])

                for i in range(NBLK):
                    xnp = xnppool.tile([128, D], F32)
                    nc.vector.tensor_scalar(
                        xnp[:], xres[:, i, :], mu[:, i : i + 1], rstd[:, i : i + 1],
                        OP.subtract, OP.mult,
                    )
                    pt = psa.tile([128, 128], F32, space="PSUM")
                    nc.tensor.transpose(pt[:], xnp[:], ident_sb[:])
                    xnpT = xnptpool.tile([128, 128], F32)
                    nc.scalar.copy(xnpT[:], pt[:])
                    pm = psb.tile([128, 129], F32, space="PSUM")
                    nc.tensor.matmul(
                        pm[:], lhsT=xnpT[:], rhs=wext_sb[:], start=True, stop=True
                    )
                    nc.vector.tensor_tensor(
                        xpstage[:, i, :], pm[:, 0:128], c2b_sb[:, 0:128], OP.add
                    )
                    nc.vector.tensor_tensor(
                        adst_all[:, i : i + 1], pm[:, 128:129], c2b_sb[:, 128:129],
                        OP.add,
                    )

            nc.sync.dma_start(
                xp_chunk.rearrange("(b p) d -> p b d", p=128)[:],
                xpstage[:],
            )

            nc.gpsimd.collective_compute(
                "AllGather",
                OP.bypass,
                replica_groups=[list(range(NCORES))],
                ins=[xp_chunk[0:SHARD, :]],
                outs=[xp_full[:, :]],
            )

            # ---------------- Phase B: edge aggregation --------------------
            with (
                tc.tile_pool(name="b_g", bufs=5) as gpool,
                tc.tile_pool(name="b_ohT", bufs=2) as ohtpool,
                tc.tile_pool(name="b_sw", bufs=2) as swpool,
                tc.tile_pool(name="b_e", bufs=3) as epool,
                tc.tile_pool(name="b_sc", bufs=2) as scpool,
                tc.tile_pool(name="b_fin", bufs=2) as fpool,
                tc.tile_pool(name="b_out", bufs=2) as opool,
                tc.tile_pool(name="b_psA", bufs=2, space="PSUM") as psA_pool,
                tc.tile_pool(name="b_psB", bufs=GBLK + 1, space="PSUM") as psB_pool,
            ):
                psBT = {}
                qn = 0
                for gi, (tiles, cb0, nlo) in enumerate(groups):
                    T = len(tiles)
                    g0 = tiles[0][0]
                    nb = tiles[-1][0] - g0 + 1   # blocks in this group
                    gf = gpool.tile([128, TG_MAX, ROW], BF16, tag="gf")
                    if nlo:
                        nc.gpsimd.dma_gather(
                            out_ap=gf[:, 0:nlo, :],
                            in_ap=xp_full[0:HALF, :],
                            idxs_ap=fidx_sb[:, cb0 * 8 : (cb0 + nlo) * 8],
                            num_idxs=nlo * 128,
                            num_idxs_reg=nlo * 128,
                            elem_size=ROW,
                            single_packet=SP,
                            queue_num=qn % NQ,
                        )
                        qn += 1
                    if T - nlo:
                        nc.gpsimd.dma_gather(
                            out_ap=gf[:, nlo:T, :],
                            in_ap=xp_full[HALF:N, :],
                            idxs_ap=fidx_sb[:, (cb0 + nlo) * 8 : (cb0 + T) * 8],
                            num_idxs=(T - nlo) * 128,
                            num_idxs_reg=(T - nlo) * 128,
                            elem_size=ROW,
                            single_packet=SP,
                            queue_num=qn % NQ,
                        )
                        qn += 1
                    ohT_sb = ohtpool.tile([128, TG_MAX * 128], BF16, tag="ohT")
                    nc.sync.dma_start(
                        ohT_sb[:, 0 : T * 128],
                        ohT_in[:, cb0 * 128 : (cb0 + T) * 128],
                    )

                    # block-major column position for each tile j in this group
                    order = sorted(range(T), key=lambda j: (tiles[j][0], tiles[j][1]))
                    bm_pos = [0] * T
                    bm_start = {}
                    for pos, j in enumerate(order):
                        bm_pos[j] = pos
                        bm_start.setdefault(tiles[j][0], pos)
                    # a_dst per slot via 1-col matmuls on the transposed one-hot;
                    # cols [TG_MAX:] hold the per-tile denominator columns
                    psA = psA_pool.tile([128, 2 * TG_MAX], F32, space="PSUM")
                    for j, (b, hf) in enumerate(tiles):
                        nc.tensor.matmul(
                            psA[:, j : j + 1],
                            lhsT=ohT_sb[:, j * 128 : (j + 1) * 128],
                            rhs=adst_all[:, b : b + 1],
                            start=True,
                            stop=True,
                        )
                    # a_src per slot: fused dot(row, att_src) via stt accum
                    asrcg = epool.tile([128, TG_MAX], F32, tag="asrcg")
                    if DEBUG_NOASRC:
                        nc.vector.memset(asrcg[:], 0.01)
                    else:
                        for j in range(T):
                            scr = scpool.tile([128, ROW], BF16, tag="scr")
                            nc.vector.scalar_tensor_tensor(
                                scr[:], gf[:, j, :], 1.0, vsrc_sb[:],
                                OP.mult, OP.mult,
                                accum_out=asrcg[:, j : j + 1],
                            )
                    e1 = epool.tile([128, TG_MAX], F32, tag="e1")
                    nc.vector.tensor_tensor(
                        e1[:, 0:T], asrcg[:, 0:T], psA[:, 0:T], OP.add
                    )
                    # ee = exp(leakyrelu(e1)); lrelu as (e1*slope) max e1 on DVE
                    e3 = epool.tile([128, TG_MAX], F32, tag="e3")
                    nc.vector.scalar_tensor_tensor(
                        e3[:, 0:T], e1[:, 0:T], NEG_SLOPE, e1[:, 0:T],
                        OP.mult, OP.max,
                    )
                    ee = epool.tile([128, TG_MAX], F32, tag="ee")
                    nc.scalar.activation(ee[:, 0:T], e3[:, 0:T], AF.Exp)

                    # sw[slot, d] = (iota == dl) * ee  (on-device one-hot)
                    sw = swpool.tile([128, TG_MAX, 128], BF16, tag="sw")
                    for j in range(T):
                        nc.vector.tensor_scalar(
                            sw[:, j, :], iota_sb[:],
                            dl_sb[:, cb0 + j : cb0 + j + 1],
                            ee[:, j : j + 1],
                            OP.is_equal, OP.mult,
                        )
                    outg = opool.tile([128, GBLK, D], F32, tag="outg")
                    for j, (b, hf) in enumerate(tiles):
                        cb = cb0 + j
                        if first_cb[b] == cb:
                            psBT[b] = psB_pool.tile(
                                [128, 128], F32, space="PSUM", tag="psBT",
                                name=f"psBT{b}",
                            )
                        nc.tensor.matmul(
                            psBT[b][:, :],
                            lhsT=sw[:, j, :],
                            rhs=gf[:, j, :],
                            start=(first_cb[b] == cb),
                            stop=(last_cb[b] == cb),
                        )
                        nc.tensor.matmul(
                            psA[:, TG_MAX + bm_pos[j] : TG_MAX + bm_pos[j] + 1],
                            lhsT=sw[:, j, :],
                            rhs=ones_sb[:],
                            start=True,
                            stop=True,
                        )
                        if last_cb[b] != cb:
                            continue
                        # -------- block finale (all on DVE) --------
                        nblk_tiles = sum(1 for (b2, _) in tiles if b2 == b)
                        dsum = fpool.tile([128, 1], F32, tag="dsum")
                        nc.vector.tensor_reduce(
                            dsum[:],
                            psA[:, TG_MAX + bm_start[b] : TG_MAX + bm_start[b] + nblk_tiles],
                            AX.X,
                            OP.add,
                        )
                        recip = fpool.tile([128, 1], F32, tag="recip")
                        nc.vector.reciprocal(recip[:], dsum[:])
                        outt = fpool.tile([128, D], F32, tag="outt")
                        nc.vector.scalar_tensor_tensor(
                            outt[:], psBT[b][:, :], recip[:, 0:1],
                            xres[:, b, :], OP.mult, OP.add,
                        )
                        nc.vector.tensor_scalar(
                            outg[:, b - g0, :], outt[:], 0.0, None, OP.max
                        )
                    nc.sync.dma_start(
                        out_shard.rearrange("(b p) d -> p b d", p=128)[
                            :, g0 : g0 + nb, :
                        ],
                        outg[:, 0:nb, :],
                    )

    nc.compile()
    return nc


def _wrap_idx(idx):
    """int16 index list -> dma_gather SBUF layout [128, len/16]."""
    L = len(idx)
    assert L % 16 == 0
    w = idx.reshape(L // 16, 16).T.astype(np.int16)
    return np.tile(w, (8, 1))


def _host_prep(x, edge_index, ln_gamma, ln_beta, W, att_src, att_dst, bias):
    """Fold parameters, bucket edges, build index tables. Numpy only."""
    Wt = W.T.astype(np.float64)
    G = ln_gamma.astype(np.float64)[:, None] * Wt          # [D, D]
    crow = ln_beta.astype(np.float64) @ Wt                 # [D]
    v_dst = G @ att_dst.astype(np.float64)
    c_dst = float(crow @ att_dst.astype(np.float64))

    wext = np.zeros((D, 129), np.float32)
    wext[:, 0:D] = G.astype(np.float32)
    wext[:, 128] = v_dst.astype(np.float32)
    c2 = np.zeros((129,), np.float32)
    c2[0:D] = (crow + bias.astype(np.float64)).astype(np.float32)
    # table rows hold xp+bias; their dot with att_src carries an extra
    # bias@att_src which we cancel inside the (src+dst)-additive constant
    c2[128] = np.float32(
        c_dst - float(bias.astype(np.float64) @ att_src.astype(np.float64))
    )
    c2b = np.broadcast_to(c2, (128, 129)).copy()
    ident = np.eye(128, dtype=np.float32)
    vsrcb = np.broadcast_to(
        att_src.astype(np.float32), (128, D)
    ).astype(ml_dtypes.bfloat16).copy()

    # edges + self loops, sorted by (core, block, src-half)
    src = np.concatenate([edge_index[0], np.arange(N, dtype=np.int64)]).astype(np.int64)
    dst = np.concatenate([edge_index[1], np.arange(N, dtype=np.int64)]).astype(np.int64)
    core = dst // SHARD
    local = dst - core * SHARD
    blk = local // 128
    half = (src >= HALF).astype(np.int64)
    key = (core * NBLK + blk) * 2 + half
    order = np.argsort(key, kind="stable")
    src, dst, key = src[order], dst[order], key[order]
    counts = np.bincount(key, minlength=NCORES * NBLK * 2).reshape(NCORES, NBLK, 2)
    tiles = -(-counts // 128)                              # ceil
    tlo = tuple(int(t) for t in tiles[:, :, 0].max(axis=0))
    thi = tuple(int(t) for t in tiles[:, :, 1].max(axis=0))
    CB = sum(tlo) + sum(thi)

    # slot offset (in tiles) of each (block, half) segment, same for all cores
    seg_off = {}
    cb0 = 0
    for g0 in range(0, NBLK, GBLK):
        blocks = list(range(g0, min(NBLK, g0 + GBLK)))
        off = cb0
        for b in blocks:
            seg_off[(b, 0)] = off
            off += tlo[b]
        for b in blocks:
            seg_off[(b, 1)] = off
            off += thi[b]
        cb0 = off
    assert cb0 == CB

    starts = np.zeros(NCORES * NBLK * 2 + 1, np.int64)
    starts[1:] = np.cumsum(counts.reshape(-1))

    feat_idx = np.zeros((NCORES, CB * 128), np.int16)
    ohT = np.zeros((NCORES, 128, CB * 128), ml_dtypes.bfloat16)
    dl_all = np.full((NCORES, 128, CB), -1.0, np.float32)

    for c in range(NCORES):
        for b in range(NBLK):
            for hf in range(2):
                gidx = (c * NBLK + b) * 2 + hf
                s, e = starts[gidx], starts[gidx + 1]
                n = int(e - s)
                if n == 0:
                    continue
                off = seg_off[(b, hf)]
                k = np.arange(n) + off * 128           # global slot ids
                fi = (src[s:e] - hf * HALF).astype(np.int16)
                feat_idx[c, k] = fi
                dl = (dst[s:e] - (c * SHARD + b * 128)).astype(np.int64)
                p = k % 128
                t = k // 128
                ohT[c, dl, t * 128 + p] = 1
                dl_all[c, p, t] = dl.astype(np.float32)

    in_maps = []
    for c in range(NCORES):
        xs = np.zeros((PAD_SHARD, D), np.float32)
        xs[0:SHARD] = x[c * SHARD : (c + 1) * SHARD]
        in_maps.append(
            {
                "x_shard": xs,
                "wext": wext,
                "c2b": c2b,
                "vsrcb": vsrcb,
                "ident": ident,
                "feat_idx": _wrap_idx(feat_idx[c]),
                "ohT_in": np.ascontiguousarray(ohT[c]),
                "dl_in": np.ascontiguousarray(dl_all[c]),
                "iota_in": np.broadcast_to(
                    np.arange(128, dtype=np.float32), (128, 128)
                ).copy(),
            }
        )
    return tlo, thi, in_maps


_PROGRAM_CACHE = {}


def kernel(x, edge_index, edge_attr, h, batch, ln_gamma, ln_beta, W, att_src,
           att_dst, bias):
    x = np.asarray(x, dtype=np.float32)
    edge_index = np.asarray(edge_index)
    h = np.asarray(h)
    ln_gamma = np.asarray(ln_gamma, dtype=np.float32)
    ln_beta = np.asarray(ln_beta, dtype=np.float32)
    W = np.asarray(W, dtype=np.float32)
    att_src = np.asarray(att_src, dtype=np.float32)
    att_dst = np.asarray(att_dst, dtype=np.float32)
    bias = np.asarray(bias, dtype=np.float32)

    tlo, thi, in_maps = _host_prep(
        x, edge_index, ln_gamma, ln_beta, W, att_src, att_dst, bias
    )
    key = (tlo, thi)
    if key not in _PROGRAM_CACHE:
        _PROGRAM_CACHE[key] = _build_program(tlo, thi)
    nc = _PROGRAM_CACHE[key]

    res = run_bass_kernel_spmd(nc, in_maps, core_ids=list(range(NCORES)))
    out = np.concatenate(
        [res.results[c]["out_shard"][:SHARD] for c in range(NCORES)], axis=0
    )
    return out, h


# revision 19
# speedup vs baseline: 1.1112x; 1.0503x over previous
"""GAT layer (LayerNorm -> GATConv(heads=1) -> residual ReLU) on 8 trn2 NeuronCores.

Sharding: destination-node parallel. Each core owns N/8 nodes, computes the
fused LN+linear transform for its shard (bf16 rows of 128 feats at 256 B
pitch plus an a_dst column per dst block), AllGathers the bf16 node table,
then processes the edges whose destination falls in its shard.

Edges are bucketed by (dst block, src half) into 128-slot tiles. Per tile:
source rows arrive via dma_gather (256 B elems, SWDGE queues round-robin),
a_dst is broadcast to slots with a 1-column matmul against a host-shipped
transposed one-hot (ohT), a_src comes from a fused dot(row, att_src) with
accumulator per tile, ee = exp(lrelu(a_src + a_dst)) and
sw = (iota == dl) * ee is built on-device in bf16 (no dense one-hot from
HBM). One matmul per tile accumulates the dst-major scatter
psBT[dst, feat] += sw^T-contract-gf, a 1-column matmul per tile drops the
softmax denominator into psA.

The per-group work is software-pipelined across emission iterations so no
engine's in-order queue head waits on same-iteration cross-engine work:
iteration it emits gather/psA/a_src/sw for group it, the scatter matmuls
for group it-1, and the fully-dependency-free finale for group it-2
(normalize, residual, ReLU on DVE).
"""

import numpy as np
import ml_dtypes

import concourse.bacc as bacc
import concourse.mybir as mybir
import concourse.tile as tile
from concourse.bass_utils import run_bass_kernel_spmd

F32 = mybir.dt.float32
BF16 = mybir.dt.bfloat16
I16 = mybir.dt.int16
AX = mybir.AxisListType
OP = mybir.AluOpType
AF = mybir.ActivationFunctionType

N = 50000
D = 128
E = 600000
NCORES = 8
SHARD = N // NCORES            # 6250
NBLK = (SHARD + 127) // 128    # 49 dst blocks per core
PAD_SHARD = NBLK * 128         # 6272
LAST_ROWS = SHARD - (NBLK - 1) * 128  # 106
ROW = 128                      # bf16 cols per table row (256 B pitch)
HALF = 32768                   # int16 index split point for the global table
NEG_SLOPE = 0.2
LN_EPS = 1e-5
GBLK = 3                       # dst blocks per group
import os as _os
NQ = int(_os.environ.get("KNQ", "4"))
DEBUG_NOASRC = _os.environ.get("DEBUG_NOASRC") == "1"  # timing probe: skip a_src


def _build_program(tlo, thi):
    """One SPMD program; per-core behaviour differs only through its inputs."""
    nc = bacc.Bacc("TRN2", num_devices=NCORES, debug=False, num_swdge_queues=NQ)

    CB = sum(tlo) + sum(thi)   # total column-block tiles per core

    x_shard = nc.dram_tensor("x_shard", [PAD_SHARD, D], F32, kind="ExternalInput")
    wext = nc.dram_tensor("wext", [D, 129], F32, kind="ExternalInput")
    c2b = nc.dram_tensor("c2b", [128, 129], F32, kind="ExternalInput")
    vsrcb = nc.dram_tensor("vsrcb", [128, 128], BF16, kind="ExternalInput")
    ident = nc.dram_tensor("ident", [128, 128], F32, kind="ExternalInput")
    feat_idx = nc.dram_tensor("feat_idx", [128, CB * 8], I16, kind="ExternalInput")
    ohT_in = nc.dram_tensor("ohT_in", [128, CB * 128], BF16, kind="ExternalInput")
    dl_in = nc.dram_tensor("dl_in", [128, CB], F32, kind="ExternalInput")
    iota_in = nc.dram_tensor("iota_in", [128, 128], BF16, kind="ExternalInput")
    out_shard = nc.dram_tensor("out_shard", [PAD_SHARD, D], F32, kind="ExternalOutput")

    # group structure: per group, lo tiles of its blocks then hi tiles
    groups = []  # (tiles, cb0, nlo) ; tiles = list of (block, half)
    cb0 = 0
    for g0 in range(0, NBLK, GBLK):
        blocks = list(range(g0, min(NBLK, g0 + GBLK)))
        tiles = []
        for b in blocks:
            tiles += [(b, 0)] * tlo[b]
        nlo = len(tiles)
        for b in blocks:
            tiles += [(b, 1)] * thi[b]
        groups.append((tiles, cb0, nlo))
        cb0 += len(tiles)
    assert cb0 == CB
    NGR = len(groups)
    TG_MAX = max(len(t) for t, _, _ in groups)

    # first/last tile index (within CB) per block, for psum start/stop
    first_cb = {}
    last_cb = {}
    for tiles, c0, _ in groups:
        for j, (b, hf) in enumerate(tiles):
            cb = c0 + j
            first_cb.setdefault(b, cb)
            last_cb[b] = cb

    # block-major denominator column position per group
    g_meta = []
    for tiles, c0, _ in groups:
        T = len(tiles)
        order = sorted(range(T), key=lambda j: (tiles[j][0], tiles[j][1]))
        bm_pos = [0] * T
        bm_start = {}
        for pos, j in enumerate(order):
            bm_pos[j] = pos
            bm_start.setdefault(tiles[j][0], pos)
        g_meta.append((bm_pos, bm_start))

    with tile.TileContext(nc) as tc:
        with (
            tc.tile_pool(name="dram", bufs=1, space="DRAM") as dram,
            tc.tile_pool(name="consts", bufs=1) as cpool,
            tc.tile_pool(name="xres", bufs=1) as xpool,
        ):
            xp_chunk = dram.tile([PAD_SHARD, ROW], BF16)
            xp_full = dram.tile([N, ROW], BF16, addr_space="Shared")

            ident_sb = cpool.tile([128, 128], F32)
            nc.sync.dma_start(ident_sb[:], ident[:, :])
            wext_sb = cpool.tile([D, 129], F32)
            nc.sync.dma_start(wext_sb[:], wext[:, :])
            c2b_sb = cpool.tile([128, 129], F32)
            nc.sync.dma_start(c2b_sb[:], c2b[:, :])
            vsrc_sb = cpool.tile([128, 128], BF16)
            nc.sync.dma_start(vsrc_sb[:], vsrcb[:, :])
            eps_sb = cpool.tile([128, 1], F32)
            nc.vector.memset(eps_sb[:], LN_EPS)
            ones_sb = cpool.tile([128, 1], BF16)
            nc.vector.memset(ones_sb[:], 1.0)
            fidx_sb = cpool.tile([128, CB * 8], I16)
            nc.sync.dma_start(fidx_sb[:], feat_idx[:, :])
            dl_sb = cpool.tile([128, CB], F32)
            nc.sync.dma_start(dl_sb[:], dl_in[:, :])
            iota_sb = cpool.tile([128, 128], BF16)
            nc.sync.dma_start(iota_sb[:], iota_in[:, :])
            adst_all = cpool.tile([128, NBLK], BF16)

            # residual x for the whole shard, one DMA
            xres = xpool.tile([128, NBLK, D], F32)
            nc.sync.dma_start(
                xres[:], x_shard.rearrange("(b p) d -> p b d", p=128)[:]
            )

            # ---------------- Phase A: node transform on own shard ---------
            with (
                tc.tile_pool(name="a_stage", bufs=1) as stgpool,
                tc.tile_pool(name="a_small", bufs=2) as spool,
                tc.tile_pool(name="a_sq", bufs=3) as sqpool,
                tc.tile_pool(name="a_xnp", bufs=4) as xnppool,
                tc.tile_pool(name="a_xnpT", bufs=4) as xnptpool,
                tc.tile_pool(name="a_ps_t", bufs=3, space="PSUM") as psa,
                tc.tile_pool(name="a_ps_m", bufs=3, space="PSUM") as psb,
            ):
                xpstage = stgpool.tile([128, NBLK, ROW], BF16)
                sumx = spool.tile([128, NBLK], F32, tag="sumx")
                nc.vector.tensor_reduce(sumx[:], xres[:], AX.X, OP.add)
                ssq = spool.tile([128, NBLK], F32, tag="ssq")
                for i in range(NBLK):
                    sqj = sqpool.tile([128, D], F32)
                    nc.scalar.activation(
                        sqj[:], xres[:, i, :], AF.Square, accum_out=ssq[:, i : i + 1]
                    )
                mu = spool.tile([128, NBLK], F32, tag="mu")
                nc.vector.tensor_scalar(mu[:], sumx[:], 1.0 / D, None, OP.mult)
                m2 = spool.tile([128, NBLK], F32, tag="m2")
                nc.vector.tensor_tensor(m2[:], mu[:], mu[:], OP.mult)
                var = spool.tile([128, NBLK], F32, tag="var")
                nc.vector.scalar_tensor_tensor(
                    var[:], ssq[:], 1.0 / D, m2[:], OP.mult, OP.subtract
                )
                std = spool.tile([128, NBLK], F32, tag="std")
                nc.scalar.activation(std[:], var[:], AF.Sqrt, bias=eps_sb[:, 0:1])
                rstd = spool.tile([128, NBLK], F32, tag="rstd")
                nc.vector.reciprocal(rstd[:], std[:])

                for i in range(NBLK):
                    xnp = xnppool.tile([128, D], F32)
                    nc.vector.tensor_scalar(
                        xnp[:], xres[:, i, :], mu[:, i : i + 1], rstd[:, i : i + 1],
                        OP.subtract, OP.mult,
                    )
                    pt = psa.tile([128, 128], F32, space="PSUM")
                    nc.tensor.transpose(pt[:], xnp[:], ident_sb[:])
                    xnpT = xnptpool.tile([128, 128], F32)
                    nc.scalar.copy(xnpT[:], pt[:])
                    pm = psb.tile([128, 129], F32, space="PSUM")
                    nc.tensor.matmul(
                        pm[:], lhsT=xnpT[:], rhs=wext_sb[:], start=True, stop=True
                    )
                    nc.vector.tensor_tensor(
                        xpstage[:, i, :], pm[:, 0:128], c2b_sb[:, 0:128], OP.add
                    )
                    nc.vector.tensor_tensor(
                        adst_all[:, i : i + 1], pm[:, 128:129], c2b_sb[:, 128:129],
                        OP.add,
                    )
                nc.sync.dma_start(
                    xp_chunk.rearrange("(b p) d -> p b d", p=128)[:],
                    xpstage[:],
                )

            nc.gpsimd.collective_compute(
                "AllGather",
                OP.bypass,
                replica_groups=[list(range(NCORES))],
                ins=[xp_chunk[0:SHARD, :]],
                outs=[xp_full[:, :]],
            )

            # ---------------- Phase B: edge aggregation --------------------
            with (
                tc.tile_pool(name="b_g", bufs=5) as gpool,
                tc.tile_pool(name="b_ohT", bufs=2) as ohtpool,
                tc.tile_pool(name="b_sw", bufs=3) as swpool,
                tc.tile_pool(name="b_e", bufs=2) as epool,
                tc.tile_pool(name="b_den", bufs=3) as denpool,
                tc.tile_pool(name="b_sc", bufs=2) as scpool,
                tc.tile_pool(name="b_fin", bufs=2) as fpool,
                tc.tile_pool(name="b_out", bufs=2) as opool,
                tc.tile_pool(name="b_psA", bufs=2, space="PSUM") as psA_pool,
                tc.tile_pool(name="b_psB", bufs=2 * GBLK, space="PSUM") as psB_pool,
            ):
                gfs, sws, psAs, dens, psBT = {}, {}, {}, {}, {}
                qn = 0
                for it in range(NGR + 2):
                    # ---- stage 0+1+2: gather, a_dst, a_src, ee, sw  (group it)
                    if it < NGR:
                        g = it
                        tiles, cbase, nlo = groups[g]
                        T = len(tiles)
                        gf = gpool.tile([128, TG_MAX, ROW], BF16, tag="gf")
                        gfs[g] = gf
                        if nlo:
                            nc.gpsimd.dma_gather(
                                out_ap=gf[:, 0:nlo, :],
                                in_ap=xp_full[0:HALF, :],
                                idxs_ap=fidx_sb[:, cbase * 8 : (cbase + nlo) * 8],
                                num_idxs=nlo * 128,
                                num_idxs_reg=nlo * 128,
                                elem_size=ROW,
                                single_packet=False,
                                queue_num=qn % NQ,
                            )
                            qn += 1
                        if T - nlo:
                            nc.gpsimd.dma_gather(
                                out_ap=gf[:, nlo:T, :],
                                in_ap=xp_full[HALF:N, :],
                                idxs_ap=fidx_sb[:, (cbase + nlo) * 8 : (cbase + T) * 8],
                                num_idxs=(T - nlo) * 128,
                                num_idxs_reg=(T - nlo) * 128,
                                elem_size=ROW,
                                single_packet=False,
                                queue_num=qn % NQ,
                            )
                            qn += 1
                        ohT_sb = ohtpool.tile([128, TG_MAX * 128], BF16, tag="ohT")
                        nc.sync.dma_start(
                            ohT_sb[:, 0 : T * 128],
                            ohT_in[:, cbase * 128 : (cbase + T) * 128],
                        )
                        # a_dst per slot via 1-col matmuls on the transposed
                        # one-hot; cols [TG_MAX:] later hold the denominators
                        psA = psA_pool.tile([128, 2 * TG_MAX], F32, space="PSUM")
                        psAs[g] = psA
                        for j, (b, hf) in enumerate(tiles):
                            nc.tensor.matmul(
                                psA[:, j : j + 1],
                                lhsT=ohT_sb[:, j * 128 : (j + 1) * 128],
                                rhs=adst_all[:, b : b + 1],
                                start=True,
                                stop=True,
                            )
                        # a_src per slot: fused dot(row, att_src) via stt accum
                        asrcg = epool.tile([128, TG_MAX], F32, tag="asrcg")
                        if DEBUG_NOASRC:
                            nc.vector.memset(asrcg[:], 0.01)
                        else:
                            for j in range(T):
                                scr = scpool.tile([128, ROW], BF16, tag="scr")
                                nc.vector.scalar_tensor_tensor(
                                    scr[:], gf[:, j, :], 1.0, vsrc_sb[:],
                                    OP.mult, OP.mult,
                                    accum_out=asrcg[:, j : j + 1],
                                )
                        e1 = epool.tile([128, TG_MAX], F32, tag="e1")
                        nc.vector.tensor_tensor(
                            e1[:, 0:T], asrcg[:, 0:T], psA[:, 0:T], OP.add
                        )
                        e3 = epool.tile([128, TG_MAX], F32, tag="e3")
                        nc.vector.scalar_tensor_tensor(
                            e3[:, 0:T], e1[:, 0:T], NEG_SLOPE, e1[:, 0:T],
                            OP.mult, OP.max,
                        )
                        ee = epool.tile([128, TG_MAX], F32, tag="ee")
                        nc.scalar.activation(ee[:, 0:T], e3[:, 0:T], AF.Exp)
                        # sw[slot, d] = (iota == dl) * ee   (all bf16)
                        sw = swpool.tile([128, TG_MAX, 128], BF16, tag="sw")
                        sws[g] = sw
                        for j in range(T):
                            nc.vector.tensor_scalar(
                                sw[:, j, :], iota_sb[:],
                                dl_sb[:, cbase + j : cbase + j + 1],
                                ee[:, j : j + 1],
                                OP.is_equal, OP.mult,
                            )
                    # ---- stage 3: scatter + denominator matmuls (group it-1)
                    if 0 <= it - 1 < NGR:
                        g = it - 1
                        tiles, cbase, nlo = groups[g]
                        T = len(tiles)
                        bm_pos, bm_start = g_meta[g]
                        gf, sw, psA = gfs[g], sws[g], psAs[g]
                        for j, (b, hf) in enumerate(tiles):
                            cb = cbase + j
                            if first_cb[b] == cb:
                                psBT[b] = psB_pool.tile(
                                    [128, 128], F32, space="PSUM", tag="psBT",
                                    name=f"psBT{b}",
                                )
                            nc.tensor.matmul(
                                psBT[b][:, :],
                                lhsT=sw[:, j, :],
                                rhs=gf[:, j, :],
                                start=(first_cb[b] == cb),
                                stop=(last_cb[b] == cb),
                            )
                            nc.tensor.matmul(
                                psA[:, TG_MAX + bm_pos[j] : TG_MAX + bm_pos[j] + 1],
                                lhsT=sw[:, j, :],
                                rhs=ones_sb[:],
                                start=True,
                                stop=True,
                            )
                        # free psA early: denominators to SBUF via ACT
                        den = denpool.tile([128, TG_MAX], F32, tag="den")
                        dens[g] = den
                        nc.scalar.copy(den[:, 0:T], psA[:, TG_MAX : TG_MAX + T])
                        del gfs[g], sws[g], psAs[g]
                    # ---- stage 4: block finales + output write (group it-2)
                    if 0 <= it - 2 < NGR:
                        g = it - 2
                        tiles, cbase, nlo = groups[g]
                        bm_pos, bm_start = g_meta[g]
                        den = dens.pop(g)
                        g0 = tiles[0][0]
                        nb = tiles[-1][0] - g0 + 1
                        outg = opool.tile([128, GBLK, D], F32, tag="outg")
                        for b in range(g0, g0 + nb):
                            nblk_tiles = sum(1 for (b2, _) in tiles if b2 == b)
                            dsum = fpool.tile([128, 1], F32, tag="dsum")
                            nc.vector.tensor_reduce(
                                dsum[:],
                                den[:, bm_start[b] : bm_start[b] + nblk_tiles],
                                AX.X,
                                OP.add,
                            )
                            recip = fpool.tile([128, 1], F32, tag="recip")
                            nc.vector.reciprocal(recip[:], dsum[:])
                            outt = fpool.tile([128, D], F32, tag="outt")
                            nc.vector.scalar_tensor_tensor(
                                outt[:], psBT[b][:, :], recip[:, 0:1],
                                xres[:, b, :], OP.mult, OP.add,
                            )
                            nc.vector.tensor_scalar(
                                outg[:, b - g0, :], outt[:], 0.0, None, OP.max
                            )
                        nc.sync.dma_start(
                            out_shard.rearrange("(b p) d -> p b d", p=128)[
                                :, g0 : g0 + nb, :
                            ],
                            outg[:, 0:nb, :],
                        )

    nc.compile()
    return nc


def _wrap_idx(idx):
    """int16 index list -> dma_gather SBUF layout [128, len/16]."""
    L = len(idx)
    assert L % 16 == 0
    w = idx.reshape(L // 16, 16).T.astype(np.int16)
    return np.tile(w, (8, 1))


def _host_prep(x, edge_index, ln_gamma, ln_beta, W, att_src, att_dst, bias):
    """Fold parameters, bucket edges, build index tables. Numpy only."""
    Wt = W.T.astype(np.float64)
    G = ln_gamma.astype(np.float64)[:, None] * Wt          # [D, D]
    crow = ln_beta.astype(np.float64) @ Wt                 # [D]
    v_dst = G @ att_dst.astype(np.float64)
    c_dst = float(crow @ att_dst.astype(np.float64))

    wext = np.zeros((D, 129), np.float32)
    wext[:, 0:D] = G.astype(np.float32)
    wext[:, 128] = v_dst.astype(np.float32)
    c2 = np.zeros((129,), np.float32)
    c2[0:D] = (crow + bias.astype(np.float64)).astype(np.float32)
    # table rows hold xp+bias; their dot with att_src carries an extra
    # bias@att_src which we cancel inside the (src+dst)-additive constant
    c2[128] = np.float32(
        c_dst - float(bias.astype(np.float64) @ att_src.astype(np.float64))
    )
    c2b = np.broadcast_to(c2, (128, 129)).copy()
    ident = np.eye(128, dtype=np.float32)
    vsrcb = np.broadcast_to(
        att_src.astype(np.float32), (128, D)
    ).astype(ml_dtypes.bfloat16).copy()

    # edges + self loops, sorted by (core, block, src-half)
    src = np.concatenate([edge_index[0], np.arange(N, dtype=np.int64)]).astype(np.int64)
    dst = np.concatenate([edge_index[1], np.arange(N, dtype=np.int64)]).astype(np.int64)
    core = dst // SHARD
    local = dst - core * SHARD
    blk = local // 128
    half = (src >= HALF).astype(np.int64)
    key = (core * NBLK + blk) * 2 + half
    order = np.argsort(key, kind="stable")
    src, dst, key = src[order], dst[order], key[order]
    counts = np.bincount(key, minlength=NCORES * NBLK * 2).reshape(NCORES, NBLK, 2)
    tiles = -(-counts // 128)                              # ceil
    tlo = tuple(int(t) for t in tiles[:, :, 0].max(axis=0))
    thi = tuple(int(t) for t in tiles[:, :, 1].max(axis=0))
    CB = sum(tlo) + sum(thi)

    # slot offset (in tiles) of each (block, half) segment, same for all cores
    seg_off = {}
    cb0 = 0
    for g0 in range(0, NBLK, GBLK):
        blocks = list(range(g0, min(NBLK, g0 + GBLK)))
        off = cb0
        for b in blocks:
            seg_off[(b, 0)] = off
            off += tlo[b]
        for b in blocks:
            seg_off[(b, 1)] = off
            off += thi[b]
        cb0 = off
    assert cb0 == CB

    starts = np.zeros(NCORES * NBLK * 2 + 1, np.int64)
    starts[1:] = np.cumsum(counts.reshape(-1))

    feat_idx = np.zeros((NCORES, CB * 128), np.int16)
    ohT = np.zeros((NCORES, 128, CB * 128), ml_dtypes.bfloat16)
    dl_all = np.full((NCORES, 128, CB), -1.0, np.float32)

    for c in range(NCORES):
        for b in range(NBLK):
            for hf in range(2):
                gidx = (c * NBLK + b) * 2 + hf
                s, e = starts[gidx], starts[gidx + 1]
                n = int(e - s)
                if n == 0:
                    continue
                off = seg_off[(b, hf)]
                k = np.arange(n) + off * 128           # global slot ids
                fi = (src[s:e] - hf * HALF).astype(np.int16)
                feat_idx[c, k] = fi
                dl = (dst[s:e] - (c * SHARD + b * 128)).astype(np.int64)
                p = k % 128
                t = k // 128
                ohT[c, dl, t * 128 + p] = 1
                dl_all[c, p, t] = dl.astype(np.float32)

    in_maps = []
    for c in range(NCORES):
        xs = np.zeros((PAD_SHARD, D), np.float32)
        xs[0:SHARD] = x[c * SHARD : (c + 1) * SHARD]
        in_maps.append(
            {
                "x_shard": xs,
                "wext": wext,
                "c2b": c2b,
                "vsrcb": vsrcb,
                "ident": ident,
                "feat_idx": _wrap_idx(feat_idx[c]),
                "ohT_in": np.ascontiguousarray(ohT[c]),
                "dl_in": np.ascontiguousarray(dl_all[c]),
                "iota_in": np.broadcast_to(
                    np.arange(128, dtype=np.float32), (128, 128)
                ).astype(ml_dtypes.bfloat16).copy(),
            }
        )
    return tlo, thi, in_maps


_PROGRAM_CACHE = {}


def kernel(x, edge_index, edge_attr, h, batch, ln_gamma, ln_beta, W, att_src,
           att_dst, bias):
    x = np.asarray(x, dtype=np.float32)
    edge_index = np.asarray(edge_index)
    h = np.asarray(h)
    ln_gamma = np.asarray(ln_gamma, dtype=np.float32)
    ln_beta = np.asarray(ln_beta, dtype=np.float32)
    W = np.asarray(W, dtype=np.float32)
    att_src = np.asarray(att_src, dtype=np.float32)
    att_dst = np.asarray(att_dst, dtype=np.float32)
    bias = np.asarray(bias, dtype=np.float32)

    tlo, thi, in_maps = _host_prep(
        x, edge_index, ln_gamma, ln_beta, W, att_src, att_dst, bias
    )
    key = (tlo, thi)
    if key not in _PROGRAM_CACHE:
        _PROGRAM_CACHE[key] = _build_program(tlo, thi)
    nc = _PROGRAM_CACHE[key]

    res = run_bass_kernel_spmd(nc, in_maps, core_ids=list(range(NCORES)))
    out = np.concatenate(
        [res.results[c]["out_shard"][:SHARD] for c in range(NCORES)], axis=0
    )
    return out, h
